# revision 19
# baseline (speedup 1.0000x reference)
"""Trainium2 Bass kernel for the GAT+HSPA cascade (nn_GAT_HSPA_Cascade).

Takes FULL inputs (B=32), shards batch across 8 NeuronCores (4 per core,
weights replicated), runs one SPMD Bass/Tile program, gathers full output.

v2 changes vs baseline (214us):
  - all BN folds / A = Wq^T Ws / u = Ws^T bq computed on HOST (numpy): kills
    the 13us Ln/Exp ACT-table thrash at startup
  - fp16 matmuls everywhere (PE runs 16-bit at ~0.9ns/row vs f32r 1.4);
    expT stays bf16 (exp range ~e^20 overflows fp16)
  - conv3x3 row-splits paired into PE column groups h0/h64 (concurrent
    64-col matmuls) -> conv time ~halves
  - gatT + zf_gT transposes moved off the PE onto the DMA XBAR
    (dma_start_transpose), killing 16 PE transposes + their PSUM evictions
  - S_sb/P in fp16 (2x DVE on max8/P production; numpy-verified 1.4e-3)
  - ACT tables (Exp/Prelu) preloaded via dummy ops during initial DMAs

Per-batch pipeline on each core (b = one image):
  azf  = (Ws^T Wq)^T zf            [256, 49]
  simT = azf^T xf                  [49, 961]
  expT = exp(simT + bq^T zf_t + SHIFT)   (bf16)
  den  = ones^T expT ; rdbc = 1/den on DVE
  xf_g = relu(bn(Wg@xf)) ; zf_g likewise
  emb  = (zf_g @ expT) * rdbc
  gat  = relu(bn(Wfi@[emb;xf_g]))  (fp16)
  gatT via DMA XBAR transpose
  e    = prelu(bn(conv3x3(gat)))   [64, 961] col-group-paired
  S    = e^T e                     [961, 961] row-group-paired fp16
  P    = sparsemax_row(S) via top-16 (max8 even/odd + bitonic merge +
         tau = max_j (cumsum_j - 1)/j), exact for support <= 15
  out  = gat @ (P + I)
"""

import numpy as np

import concourse.bass as bass
import concourse.mybir as mybir
from concourse import bacc
from concourse.tile import TileContext
from concourse.masks import make_identity
from concourse.bass_utils import run_bass_kernel_spmd

F32 = mybir.dt.float32
F16 = mybir.dt.float16
BF16 = mybir.dt.bfloat16
AF = mybir.ActivationFunctionType
ALU = mybir.AluOpType

# problem constants (hardcoded per contract)
B, C, CM = 32, 256, 64
HX, WX, HZ, WZ = 31, 31, 7, 7
NX, NZ = HX * WX, HZ * WZ          # 961, 49
NCORES = 8
BL = B // NCORES                   # 4 batches per core
EPS = 1e-5
PW = HX + 2                        # padded spatial height 33
EXP_SHIFT = -40.0                  # constant softmax shift (sim |max| ~ 60)

# flat splits of the 961-wide free dim, 512-aligned for PSUM banks
FSPLITS = [(0, 512), (512, 449)]
# conv row-splits -> (row0, nrows, psum/ef partition offset); col groups
CSPLITS = [(0, 16, 0), (16, 15, 64)]
CB = 16 * WX                       # 496: col boundary between the splits
PWW = 34  # padded gat buffer row width (31 + 1 left + 2 right)
# 961 rows -> 8 partition chunks
NCH = [(ci * 128, min(128, NX - ci * 128)) for ci in range(8)]
GSB_W = 1024  # gat_sb padded width


def build_bass():
    nc = bacc.Bacc(None, target_bir_lowering=False)

    # ---------------- DRAM I/O ----------------
    # weights arrive in kernel-friendly layouts, BN already folded (host)
    zf_h = nc.dram_tensor("zf", [128, 2, BL, NZ], F16, kind="ExternalInput")
    xf_h = nc.dram_tensor("xf", [BL, C, HX, WX], F16, kind="ExternalInput")
    AT_h = nc.dram_tensor("AT_nat", [128, 2, C], F16, kind="ExternalInput")
    u2_h = nc.dram_tensor("u2", [128, 2, 2], F16, kind="ExternalInput")
    WgT_h = nc.dram_tensor("WgT", [128, 2, C], F16, kind="ExternalInput")
    WfiT_h = nc.dram_tensor("WfiT", [128, 4, C], F16, kind="ExternalInput")
    WmT_h = nc.dram_tensor("WmT", [128, 2, 9, CM], F16, kind="ExternalInput")
    g_s_h = nc.dram_tensor("g_s", [C], F32, kind="ExternalInput")
    g_b_h = nc.dram_tensor("g_b", [C], F32, kind="ExternalInput")
    fi_s_h = nc.dram_tensor("fi_s", [C], F32, kind="ExternalInput")
    fi_b_h = nc.dram_tensor("fi_b", [C], F32, kind="ExternalInput")
    m_s_h = nc.dram_tensor("m_s", [128], F32, kind="ExternalInput")
    m_b_h = nc.dram_tensor("m_b", [128], F32, kind="ExternalInput")
    prelu_h = nc.dram_tensor("prelu_a", [1], F32, kind="ExternalInput")
    out_h = nc.dram_tensor("out", [BL, C, HX, WX], F32, kind="ExternalOutput")

    from contextlib import ExitStack
    with TileContext(nc) as tc, ExitStack() as ctx:
        wpool = ctx.enter_context(tc.tile_pool(name="weights", bufs=1))
        apool = ctx.enter_context(tc.tile_pool(name="acts", bufs=1))
        dbl = ctx.enter_context(tc.tile_pool(name="dbl", bufs=2))
        spool = ctx.enter_context(tc.tile_pool(name="smat", bufs=1))
        vpool = ctx.enter_context(tc.tile_pool(name="vsmall", bufs=2))
        pp = ctx.enter_context(tc.tile_pool(name="ps_big", bufs=3, space="PSUM"))
        pps = ctx.enter_context(tc.tile_pool(name="ps_small", bufs=2, space="PSUM"))

        # register constant bias APs used by scalar.activation float biases
        kt0 = wpool.tile([128, 1], F32, tag="konst_0")
        nc.vector.memset(kt0, 0.0)
        nc.const_aps.aps[(F32, 0.0)] = kt0[:]

        # preload ACT tables (Exp for softmax, Prelu for conv) with dummy ops
        # so the 1.5us table loads happen during the initial weight DMAs
        dumm = wpool.tile([128, 1], F32, tag="dummy")
        nc.scalar.activation(dumm, kt0, AF.Exp)
        nc.scalar.activation(dumm, kt0, AF.Prelu, alpha=kt0)
        nc.scalar.activation(dumm, kt0, AF.Relu)

        # ---------------- load weights (host-prepped layouts) -------------
        AT_sb = wpool.tile([128, 2, C], F16, tag="AT")
        nc.sync.dma_start(out=AT_sb, in_=AT_h[:])
        u2 = wpool.tile([128, 2, 2], F16, tag="u2")
        nc.sync.dma_start(out=u2, in_=u2_h[:])
        WgT = wpool.tile([128, 2, C], F16, tag="wgT")
        nc.sync.dma_start(out=WgT, in_=WgT_h[:])

        def load_vec(h, n, name):
            ch = max(n // 128, 1)
            p = min(n, 128)
            t = wpool.tile([p, ch], F32, tag=f"v_{name}")
            nc.sync.dma_start(out=t, in_=h[:].rearrange("(a p) -> p a", p=p))
            return t

        g_s = load_vec(g_s_h, C, "g_s")
        g_b = load_vec(g_b_h, C, "g_b")
        fi_s = load_vec(fi_s_h, C, "fi_s")
        fi_b = load_vec(fi_b_h, C, "fi_b")
        m_s = load_vec(m_s_h, 128, "m_s")
        m_b = load_vec(m_b_h, 128, "m_b")
        prelu_sb = wpool.tile([128, 1], F32, tag="prelu")
        nc.sync.dma_start(out=prelu_sb,
                          in_=prelu_h[:].unsqueeze(0).to_broadcast([128, 1]))

        # ---------------- hoisted zf stage (all BL batches at once) --------
        # zf is tiny (49 cols/batch): run azf/zf_g/zf_gT for all 4 batches in
        # one go at startup -> fewer small matmuls + fewer fixed-cost ACT ops
        zf_all = wpool.tile([128, 2, BL, NZ], F16, tag="zf_all")
        nc.sync.dma_start(out=zf_all, in_=zf_h[:])
        azf_all = wpool.tile([128, 2, BL, NZ], F16, tag="azf_all")
        zfg_all = wpool.tile([128, 2, BL, 128], BF16, tag="zfg_all")
        zfgT_all = wpool.tile([128, 2, BL, 128], BF16, tag="zfgT_all")

        def emit_zf_stage():
            ps_az = pps.tile([128, 2, BL * NZ], F32, tag="psmall")
            for ih in range(2):
                for jc in range(2):
                    nc.tensor.matmul(ps_az[:, ih, :],
                                     AT_sb[:, jc, ih * 128:(ih + 1) * 128],
                                     zf_all[:, jc], start=(jc == 0),
                                     stop=(jc == 1))
            nc.scalar.activation(azf_all, ps_az, AF.Copy)

            nc.vector.memset(zfg_all, 0.0)
            for a in range(2):
                ps_g = pps.tile([128, BL * NZ], F32, tag="psmall")
                for jc in range(2):
                    nc.tensor.matmul(ps_g, WgT[:, jc, a * 128:(a + 1) * 128],
                                     zf_all[:, jc], start=(jc == 0),
                                     stop=(jc == 1))
                nc.scalar.activation(
                    zfg_all[:, a, :, :NZ],
                    ps_g.rearrange("p (b j) -> p b j", j=NZ), AF.Relu,
                    bias=g_b[:, a:a + 1], scale=g_s[:, a:a + 1])
            nc.sync.dma_start_transpose(
                out=zfgT_all.rearrange("p a b j -> p (a b) j"),
                in_=zfg_all.rearrange("p a b j -> p (a b j)"))

        # ---------------- software-pipelined per-batch stages ----------------
        st = [dict() for _ in range(BL)]

        def emit_load(b):
            s = st[b]
            xf_r = dbl.tile([128, 2, NX], F16, tag="xf_r", name="xf_r")
            nc.sync.dma_start(
                out=xf_r,
                in_=xf_h[b].rearrange("(a p) h w -> p a (h w)", p=128))
            s['xf_r'] = xf_r

        def emit_front_p1(b):
            s = st[b]
            xf_r = s['xf_r']

            # term2[m] = u . zf[:, m]  -> exp bias row [49, 1]
            ps_t2 = pps.tile([NZ, 2], F32, tag="psmall")
            for jc in range(2):
                nc.tensor.matmul(ps_t2, zf_all[:, jc, b, :], u2[:, jc, :],
                                 start=(jc == 0), stop=(jc == 1))
            exp_bias = vpool.tile([NZ, 1], F32, tag="exp_bias", name="exp_bias")
            nc.vector.tensor_scalar(exp_bias, ps_t2[:, 0:1], EXP_SHIFT, None,
                                    ALU.add)

            # simT = azf^T xf  [49, 961] fp16
            ps_sT = pp.tile([128, 1024], F32, tag="mm961")
            for f0, fw in FSPLITS:
                sl = slice(f0, f0 + fw)
                for k in range(2):
                    nc.tensor.matmul(ps_sT[:NZ, sl], azf_all[:, k, b, :],
                                     xf_r[:, k, sl], start=(k == 0), stop=(k == 1))
            expT = apool.tile([NZ, NX], BF16, tag="expT")
            nc.scalar.activation(expT, ps_sT[:NZ, :NX], AF.Exp,
                                 bias=exp_bias)

            # -- g path fills the PE while exp cooks --
            xf_g = apool.tile([128, 2, NX], F16, tag="xf_g")
            for a in range(2):
                ps_g = pp.tile([128, 1024], F32, tag="mm961")
                for f0, fw in FSPLITS:
                    sl = slice(f0, f0 + fw)
                    for k in range(2):
                        nc.tensor.matmul(
                            ps_g[:, sl], WgT[:, k, a * 128:(a + 1) * 128],
                            xf_r[:, k, sl],
                            start=(k == 0), stop=(k == 1))
                nc.scalar.activation(xf_g[:, a, :], ps_g[:, :NX], AF.Relu,
                                     bias=g_b[:, a:a + 1], scale=g_s[:, a:a + 1])

            s['expT'] = expT
            s['xf_g'] = xf_g

        def emit_front_p2(b):
            s = st[b]
            expT, xf_g = s['expT'], s['xf_g']

            # den broadcast to all 128 partitions via ones[49,128] lhsT
            ps_den = pp.tile([128, 1024], F32, tag="mm961")
            for f0, fw in FSPLITS:
                sl = slice(f0, f0 + fw)
                nc.tensor.matmul(ps_den[:, sl], ones49, expT[:, sl],
                                 start=True, stop=True)
            # rdbc = 1/den on DVE (no Ln/Exp -> act table stays put)
            rdbc = apool.tile([128, NX], F32, tag="rdbc")
            nc.vector.reciprocal_approx_fast(out=rdbc, in_=ps_den[:, :NX])
            # normalized attention weights on GPSIMD (SBUF-only op)
            expT_s = apool.tile([NZ, NX], BF16, tag="expT_s")
            nc.gpsimd.tensor_mul(expT_s, expT, rdbc[:NZ, :])

            # emb = zf_g @ (expT/den)  [256, 961]
            emb = apool.tile([128, 2, NX], F16, tag="emb")
            for a in range(2):
                ps = pp.tile([128, 1024], F32, tag="mm961")
                for f0, fw in FSPLITS:
                    sl = slice(f0, f0 + fw)
                    nc.tensor.matmul(ps[:, sl],
                                     zfgT_all[:NZ, a, b, :],
                                     expT_s[:, sl], start=True, stop=True)
                nc.vector.tensor_copy(emb[:, a, :], ps[:, :NX])

            # gat: fi matmul -> relu(bn) -> fp16 contiguous gat_sb
            gat_sb = dbl.tile([128, 2, GSB_W], F16, tag="gat_sb")
            nc.vector.memset(gat_sb[:, :, NX:], 0.0)
            s['gat_sb'] = gat_sb
            gat_pad = gat_pads[b % 2]
            s['gat_pad'] = gat_pad
            for a in range(2):
                ps = pp.tile([128, 1024], F32, tag="mm961")
                for f0, fw in FSPLITS:
                    sl = slice(f0, f0 + fw)
                    for k in range(4):
                        rhs = emb[:, k, sl] if k < 2 else xf_g[:, k - 2, sl]
                        nc.tensor.matmul(ps[:, sl],
                                         WfiT[:, k, a * 128:(a + 1) * 128],
                                         rhs, start=(k == 0), stop=(k == 3))
                nc.scalar.activation(gat_sb[:, a, :NX], ps[:, :NX], AF.Relu,
                                     bias=fi_b[:, a:a + 1], scale=fi_s[:, a:a + 1])
                # padded copy for the conv windows (gpsimd: SBUF->SBUF fp16)
                nc.gpsimd.tensor_copy(
                    gat_pad[:, a, 1:1 + HX, 1:1 + WX],
                    gat_sb[:, a, :NX].rearrange("p (h w) -> p h w", w=WX))

        def emit_conv(b):
            s = st[b]
            gat_pad = s['gat_pad']
            # e = prelu(bn(conv3x3(gat))) -> ef [128, 961] fp16
            # the two row-splits run in PE column groups h0/h64 concurrently
            ef = apool.tile([128, NX], F16, tag="ef")
            ps_e = pp.tile([128, 1024], F32, tag="mm961")
            first = {0: True, 64: True}
            for t in range(9):
                dy, dx = t // 3, t % 3
                for k in range(2):
                    for r0, nr, po in CSPLITS:
                        nc.tensor.matmul(
                            ps_e[po:po + CM, :nr * 32], WmT[:, k, t, :],
                            gat_pad[:, k, dy + r0:dy + r0 + nr, dx:dx + 32],
                            start=first[po], stop=(t == 8 and k == 1))
                        first[po] = False
            # e = prelu(m_s*conv + m_b) in one ACT op per split; each split's
            # cross-half replica DMA starts right after its own eviction
            for r0, nr, po in CSPLITS:
                pv = ps_e[po:po + CM, :nr * 32].rearrange(
                    "p (r w) -> p r w", w=32)[:, :, :WX]
                ov = slice(r0 * WX, (r0 + nr) * WX)
                nc.scalar.activation(
                    ef[po:po + CM, ov].rearrange("p (r w) -> p r w", w=WX), pv,
                    AF.Prelu, bias=m_b[po:po + CM], scale=m_s[po:po + CM],
                    alpha=prelu_sb[po:po + CM])
                # gpsimd-issued DMA: keeps the sync queue free for transposes
                if po == 0:
                    nc.gpsimd.dma_start(out=ef[CM:128, :CB], in_=ef[:CM, :CB])
                else:
                    nc.gpsimd.dma_start(out=ef[:CM, CB:NX], in_=ef[CM:128, CB:NX])
            s['ef'] = ef

            # gatT (n-major, fp16) via DMA XBAR transpose (off the PE);
            # all 16 128x128 blocks batched into ONE instruction
            gat_sb = s['gat_sb']
            gatT = dbl.tile([128, 2, 8, 128], F16, tag="gatT")
            s['gatT'] = gatT
            nc.sync.dma_start_transpose(
                out=gatT.rearrange("p a ci j -> p (a ci) j"),
                in_=gat_sb.rearrange("p a j -> p (a j)"))

        def emit_S(b, prange=(0, 4), last=False):
            s = st[b]
            ef = s['ef']
            # S = ef^T ef  [961, 961] fp16; chunk pairs run concurrently in
            # PE row-groups h0/h64 (K=64 each)
            if prange[0] == 0:
                s['S_sb'] = spool.tile([128, 8, NX], F16, tag="S", name="S_sb")
                s['v8'] = vpool.tile([128, 8, 8], F32, tag="v8", name="v8")
                s['tau'] = vpool.tile([128, 8], F32, tag="tau", name="tau_t")
                s['ntau'] = vpool.tile([128, 8], F32, tag="ntau", name="ntau_t")
            S_sb, v8 = s['S_sb'], s['v8']
            for p in range(*prange):
                c0, c1 = 2 * p, 2 * p + 1
                n00, cs0 = NCH[c0]
                n01, cs1 = NCH[c1]
                psA = pp.tile([128, 1024], F32, tag="mm961")
                psB = pp.tile([128, 1024], F32, tag="mm961")
                for f0, fw in FSPLITS:
                    sl = slice(f0, f0 + fw)
                    nc.tensor.matmul(psA[:cs0, sl], ef[:CM, n00:n00 + cs0],
                                     ef[:CM, sl], start=True, stop=True)
                    nc.tensor.matmul(psB[:cs1, sl],
                                     ef[CM:128, n01:n01 + cs1],
                                     ef[CM:128, sl], start=True, stop=True)
                if last:
                    # tail mode: top-8 straight from PSUM, evictions via ACT
                    for cc, css, pst in ((c0, cs0, psA), (c1, cs1, psB)):
                        nc.vector.max(out=v8[:css, cc, :], in_=pst[:css, :NX])
                    nc.scalar.activation(S_sb[:cs0, c0, :], psA[:cs0, :NX],
                                         AF.Copy)
                    nc.scalar.activation(S_sb[:cs1, c1, :], psB[:cs1, :NX],
                                         AF.Copy)
                else:
                    nc.vector.tensor_copy(S_sb[:cs0, c0, :], psA[:cs0, :NX])
                    nc.scalar.activation(S_sb[:cs1, c1, :], psB[:cs1, :NX],
                                         AF.Copy)
                    for cc, css in ((c0, cs0), (c1, cs1)):
                        nc.vector.max(out=v8[:css, cc, :],
                                      in_=S_sb[:css, cc, :])

        def emit_topk(b, chunks=slice(0, 8)):
            # cumsum + tau for a chunk range (v8 already sorted descending)
            s = st[b]
            nch = chunks.stop - chunks.start
            v8 = s['v8'][:, chunks, :]
            eng = nc.vector
            mA_t = vpool.tile([128, 8, 8], F32, tag="mA", name="mA_t")
            mB_t = vpool.tile([128, 8, 8], F32, tag="mB", name="mB_t")
            mA = mA_t[:, chunks, :]
            mB = mB_t[:, chunks, :]
            # Hillis-Steele cumsum over 8
            eng.tensor_tensor(mA[:, :, 1:8], v8[:, :, 1:8],
                              v8[:, :, 0:7], ALU.add)
            eng.tensor_copy(mA[:, :, 0:1], v8[:, :, 0:1])
            eng.tensor_tensor(mB[:, :, 2:8], mA[:, :, 2:8],
                              mA[:, :, 0:6], ALU.add)
            eng.tensor_copy(mB[:, :, 0:2], mA[:, :, 0:2])
            eng.tensor_tensor(mA[:, :, 4:8], mB[:, :, 4:8],
                              mB[:, :, 0:4], ALU.add)
            eng.tensor_copy(mA[:, :, 0:4], mB[:, :, 0:4])
            # t_j = (cumsum_j - 1) / j ; tau = max_j t_j
            nc.vector.scalar_tensor_tensor(mB, mA, 1.0,
                                            rj.to_broadcast([128, nch, 8]),
                                            op0=ALU.subtract, op1=ALU.mult)
            tau, ntau = s['tau'], s['ntau']
            eng.tensor_reduce(tau[:, chunks], mB, mybir.AxisListType.X, ALU.max)
            nc.vector.tensor_scalar(ntau[:, chunks], tau[:, chunks], -1.0,
                                    None, ALU.mult)

        def emit_P_chunk(s, ci, P_sb):
            # SBUF->SBUF: even chunks on DVE, odd chunks on GPSIMD
            S_sb, tau, ntau = s['S_sb'], s['tau'], s['ntau']
            n0, cs = NCH[ci]
            eng = nc.vector if ci % 2 == 0 else nc.gpsimd
            eng.tensor_scalar(P_sb[:cs, ci, :], S_sb[:cs, ci, :],
                              tau[:cs, ci:ci + 1], 0.0,
                              ALU.subtract, ALU.max)
            # fold the "+gat" residual into P: out = gat @ (P + I)
            eng.tensor_tensor(P_sb[:cs, ci, n0:n0 + cs],
                              P_sb[:cs, ci, n0:n0 + cs],
                              ident_h[:cs, :cs], ALU.add)

        def emit_P(b):
            # P = relu(S - tau) in fp16, split DVE/ACT; emitted late so the
            # ACT chunks don't head-of-line-block the next batch's copies
            s = st[b]
            P_sb = spool.tile([128, 8, NX], F16, tag="P")
            s['P_sb'] = P_sb
            for ci in range(8):
                emit_P_chunk(s, ci, P_sb)

        def emit_out(b):
            s = st[b]
            gatT, P_sb = s['gatT'], s['P_sb']
            out_sb = dbl.tile([128, 2, NX], F32, tag="out_sb")
            for a in range(2):
                ps = pp.tile([128, 1024], F32, tag="mm961")
                for f0, fw in FSPLITS:
                    sl = slice(f0, f0 + fw)
                    for ci, (n0, cs) in enumerate(NCH):
                        nc.tensor.matmul(ps[:, sl],
                                         gatT[:cs, a, ci, :],
                                         P_sb[:cs, ci, sl],
                                         start=(ci == 0), stop=(ci == 7))
                nc.scalar.activation(out_sb[:, a, :], ps[:, :NX], AF.Copy)
            nc.sync.dma_start(
                out=out_h[b].rearrange("(a p) h w -> p a (h w)", p=128),
                in_=out_sb)

        def emit_out_tail(b):
            # last batch: interleave P chunk production with the
            # PSUM-accumulating out matmuls so the PE starts immediately
            s = st[b]
            gatT = s['gatT']
            P_sb = spool.tile([128, 8, NX], F16, tag="P")
            s['P_sb'] = P_sb
            ps0 = pp.tile([128, 1024], F32, tag="mm961")
            ps1 = pp.tile([128, 1024], F32, tag="mm961")
            pss = (ps0, ps1)
            for ci, (n0, cs) in enumerate(NCH):
                emit_P_chunk(s, ci, P_sb)
                for a in range(2):
                    for f0, fw in FSPLITS:
                        sl = slice(f0, f0 + fw)
                        nc.tensor.matmul(pss[a][:, sl],
                                         gatT[:cs, a, ci, :],
                                         P_sb[:cs, ci, sl],
                                         start=(ci == 0), stop=(ci == 7))
            out_sb = dbl.tile([128, 2, NX], F32, tag="out_sb")
            for a in range(2):
                nc.scalar.activation(out_sb[:, a, :], pss[a][:, :NX], AF.Copy)
                nc.sync.dma_start(
                    out=out_h[b].rearrange("(a p) h w -> p a (h w)",
                                           p=128)[:, a:a + 1, :],
                    in_=out_sb[:, a:a + 1, :])

        emit_load(0)
        emit_zf_stage()
        emit_front_p1(0)

        # ones [49, 128] bf16: den broadcast matmul lhsT (M=128)
        ones49 = wpool.tile([NZ, 128], BF16, tag="ones49")
        nc.vector.memset(ones49, 1.0)

        # identity (fp16) for the sparsemax residual fold
        ident_f32 = wpool.tile([128, 128], F32, tag="ident_f32")
        make_identity(nc, ident_f32)
        ident_h = wpool.tile([128, 128], F16, tag="ident_h")
        nc.vector.tensor_copy(ident_h, ident_f32)

        # 1/j constants for the sparsemax threshold, j = 1..16
        rj = wpool.tile([128, 1, 8], F32, tag="rj")
        for j in range(8):
            nc.vector.memset(rj[:, :, j:j + 1], 1.0 / (j + 1))

        # persistent padded gat buffers (fp16), borders zeroed once
        gat_pads = []
        for par in range(2):
            gp = wpool.tile([128, 2, PW, PWW], F16, tag=f"gat_pad{par}")
            for a in range(2):
                nc.vector.memset(gp[:, a, 0, :], 0.0)
                nc.vector.memset(gp[:, a, PW - 1, :], 0.0)
                nc.vector.memset(gp[:, a, 1:PW - 1, 0:1], 0.0)
                nc.vector.memset(gp[:, a, 1:PW - 1, 1 + WX:PWW], 0.0)
            gat_pads.append(gp)
        # bulk weights arrive after the small/startup-critical loads
        WfiT = wpool.tile([128, 4, C], F16, tag="wfiT")
        nc.sync.dma_start(out=WfiT, in_=WfiT_h[:])
        WmT = wpool.tile([128, 2, 9, CM], F16, tag="wmT")
        nc.sync.dma_start(out=WmT, in_=WmT_h[:])

        emit_front_p2(0)
        emit_conv(0)
        emit_S(0)
        for b in range(1, BL):
            last = (b == BL - 1)
            emit_load(b)
            emit_topk(b - 1)
            emit_front_p1(b)
            emit_front_p2(b)
            emit_conv(b)
            emit_P(b - 1)
            emit_out(b - 1)
            if last:
                emit_S(b, (0, 2), last=True)
                emit_topk(b, slice(0, 4))
                emit_S(b, (2, 4), last=True)
                emit_topk(b, slice(4, 8))
            else:
                emit_S(b)
        emit_out_tail(BL - 1)

    nc.compile()
    return nc


_CACHED = None


def _get_nc():
    global _CACHED
    if _CACHED is None:
        _CACHED = build_bass()
    return _CACHED


def make_in_maps(inputs):
    """Host-side layout prep + BN folds + batch sharding."""
    full = {k: np.ascontiguousarray(np.asarray(v, dtype=np.float32))
            for k, v in inputs.items()}
    w = {}
    # A = Wq^T Ws fold: azf = A @ zf via lhsT AT[j, i] = sum_o Ws[o,j] Wq[o,i]
    AT = full['Ws'].T @ full['Wq']                       # [j, i] = [256, 256]
    w['AT_nat'] = np.ascontiguousarray(
        AT.reshape(2, 128, C).transpose(1, 0, 2)).astype(np.float16)
    u = full['Ws'].T @ full['bq']                        # [256]
    u_r = u.reshape(2, 128).T                            # [128, 2]
    w['u2'] = np.ascontiguousarray(
        np.repeat(u_r[:, :, None], 2, axis=2)).astype(np.float16)
    w['WgT'] = np.ascontiguousarray(
        full['Wg'].reshape(C, 2, 128).transpose(2, 1, 0)).astype(np.float16)
    w['WfiT'] = np.ascontiguousarray(
        full['Wfi'].reshape(C, 4, 128).transpose(2, 1, 0)).astype(np.float16)
    w['WmT'] = np.ascontiguousarray(
        full['Wm'].reshape(CM, 2, 128, 9).transpose(2, 1, 3, 0)).astype(np.float16)

    # BN folds: y = s*x_conv + b, with s = gamma/sqrt(var+eps),
    # b = beta - mean*s + s*conv_bias
    def fold(gamma, beta, mean, var, conv_b):
        s = gamma / np.sqrt(var + EPS)
        return s.astype(np.float32), (beta - mean * s + s * conv_b).astype(np.float32)

    w['g_s'], w['g_b'] = fold(full['g_gamma'], full['g_beta'],
                              full['g_mean'], full['g_var'], full['bg'])
    w['fi_s'], w['fi_b'] = fold(full['fi_gamma'], full['fi_beta'],
                                full['fi_mean'], full['fi_var'], full['bfi'])
    m_s, m_b = fold(full['m_gamma'], full['m_beta'],
                    full['m_mean'], full['m_var'], full['bm'])
    w['m_s'] = np.tile(m_s, 2)                            # both psum halves
    w['m_b'] = np.tile(m_b, 2)
    w['prelu_a'] = full['prelu_a'].reshape(1)
    zf16 = full['zf'].astype(np.float16)
    xf16 = full['xf'].astype(np.float16)
    in_maps = []
    for c in range(NCORES):
        m = dict(w)
        zc = zf16[c * BL:(c + 1) * BL]          # [BL, C, HZ, WZ]
        m['zf'] = np.ascontiguousarray(
            zc.reshape(BL, 2, 128, NZ).transpose(2, 1, 0, 3))
        m['xf'] = xf16[c * BL:(c + 1) * BL]
        in_maps.append(m)
    return in_maps


def kernel(**inputs):
    nc = _get_nc()
    in_maps = make_in_maps(inputs)
    res = run_bass_kernel_spmd(nc, in_maps, core_ids=list(range(NCORES)))
    out = np.concatenate([r['out'] for r in res.results], axis=0)
    return out.astype(np.float32)


if __name__ == "__main__":
    # smoke-build
    nc = build_bass()
    print("built ok:",
          sum(len(b.instructions) for f in nc.m.functions for b in f.blocks),
          "instructions")


# revision 21
# speedup vs baseline: 2.1643x; 2.1643x over previous
"""Trainium2 Bass kernel for the GAT+HSPA cascade (nn_GAT_HSPA_Cascade).

Takes FULL inputs (B=32), shards batch across 8 NeuronCores (4 per core,
weights replicated), runs one SPMD Bass/Tile program, gathers full output.

v2 changes vs baseline (214us):
  - all BN folds / A = Wq^T Ws / u = Ws^T bq computed on HOST (numpy): kills
    the 13us Ln/Exp ACT-table thrash at startup
  - fp16 matmuls everywhere (PE runs 16-bit at ~0.9ns/row vs f32r 1.4);
    expT stays bf16 (exp range ~e^20 overflows fp16)
  - conv3x3 row-splits paired into PE column groups h0/h64 (concurrent
    64-col matmuls) -> conv time ~halves
  - gatT + zf_gT transposes moved off the PE onto the DMA XBAR
    (dma_start_transpose), killing 16 PE transposes + their PSUM evictions
  - S_sb/P in fp16 (2x DVE on max8/P production; numpy-verified 1.4e-3)
  - ACT tables (Exp/Prelu) preloaded via dummy ops during initial DMAs

Per-batch pipeline on each core (b = one image):
  azf  = (Ws^T Wq)^T zf            [256, 49]
  simT = azf^T xf                  [49, 961]
  expT = exp(simT + bq^T zf_t + SHIFT)   (bf16)
  den  = ones^T expT ; rdbc = 1/den on DVE
  xf_g = relu(bn(Wg@xf)) ; zf_g likewise
  emb  = (zf_g @ expT) * rdbc
  gat  = relu(bn(Wfi@[emb;xf_g]))  (fp16)
  gatT via DMA XBAR transpose
  e    = prelu(bn(conv3x3(gat)))   [64, 961] col-group-paired
  S    = e^T e                     [961, 961] row-group-paired fp16
  P    = sparsemax_row(S) via top-16 (max8 even/odd + bitonic merge +
         tau = max_j (cumsum_j - 1)/j), exact for support <= 15
  out  = gat @ (P + I)
"""

import numpy as np

import concourse.bass as bass
import concourse.mybir as mybir
from concourse import bacc
from concourse.tile import TileContext
from concourse.masks import make_identity
from concourse.bass_utils import run_bass_kernel_spmd

F32 = mybir.dt.float32
F16 = mybir.dt.float16
BF16 = mybir.dt.bfloat16
AF = mybir.ActivationFunctionType
ALU = mybir.AluOpType

# problem constants (hardcoded per contract)
B, C, CM = 32, 256, 64
HX, WX, HZ, WZ = 31, 31, 7, 7
NX, NZ = HX * WX, HZ * WZ          # 961, 49
NCORES = 8
BL = B // NCORES                   # 4 batches per core
EPS = 1e-5
PW = HX + 2                        # padded spatial height 33
EXP_SHIFT = -40.0                  # constant softmax shift (sim |max| ~ 60)

# flat splits of the 961-wide free dim, 512-aligned for PSUM banks
FSPLITS = [(0, 512), (512, 449)]
# conv row-splits -> (row0, nrows, psum/ef partition offset); col groups
CSPLITS = [(0, 16, 0), (16, 15, 64)]
CB = 16 * WX                       # 496: col boundary between the splits
PWW = 34  # padded gat buffer row width (31 + 1 left + 2 right)
# 961 rows -> 8 partition chunks
NCH = [(ci * 128, min(128, NX - ci * 128)) for ci in range(8)]
GSB_W = 1024  # gat_sb padded width


def build_bass():
    nc = bacc.Bacc(None, target_bir_lowering=False)

    # ---------------- DRAM I/O ----------------
    # weights arrive in kernel-friendly layouts, BN already folded (host)
    zf_h = nc.dram_tensor("zf", [128, 2, BL, NZ], F16, kind="ExternalInput")
    xf_h = nc.dram_tensor("xf", [BL, C, HX, WX], F16, kind="ExternalInput")
    AT_h = nc.dram_tensor("AT_nat", [128, 2, C], F16, kind="ExternalInput")
    u2_h = nc.dram_tensor("u2", [128, 2, 2], F16, kind="ExternalInput")
    WgT_h = nc.dram_tensor("WgT", [128, 2, C], F16, kind="ExternalInput")
    WfiT_h = nc.dram_tensor("WfiT", [128, 4, C], F16, kind="ExternalInput")
    WmT_h = nc.dram_tensor("WmT", [128, 2, 9, CM], F16, kind="ExternalInput")
    g_s_h = nc.dram_tensor("g_s", [C], F32, kind="ExternalInput")
    g_b_h = nc.dram_tensor("g_b", [C], F32, kind="ExternalInput")
    fi_s_h = nc.dram_tensor("fi_s", [C], F32, kind="ExternalInput")
    fi_b_h = nc.dram_tensor("fi_b", [C], F32, kind="ExternalInput")
    m_s_h = nc.dram_tensor("m_s", [128], F32, kind="ExternalInput")
    m_b_h = nc.dram_tensor("m_b", [128], F32, kind="ExternalInput")
    prelu_h = nc.dram_tensor("prelu_a", [1], F32, kind="ExternalInput")
    out_h = nc.dram_tensor("out", [BL, C, HX, WX], F32, kind="ExternalOutput")

    from contextlib import ExitStack
    with TileContext(nc) as tc, ExitStack() as ctx:
        wpool = ctx.enter_context(tc.tile_pool(name="weights", bufs=1))
        apool = ctx.enter_context(tc.tile_pool(name="acts", bufs=1))
        dbl = ctx.enter_context(tc.tile_pool(name="dbl", bufs=2))
        spool = ctx.enter_context(tc.tile_pool(name="smat", bufs=1))
        vpool = ctx.enter_context(tc.tile_pool(name="vsmall", bufs=2))
        pp = ctx.enter_context(tc.tile_pool(name="ps_big", bufs=3, space="PSUM"))
        pps = ctx.enter_context(tc.tile_pool(name="ps_small", bufs=2, space="PSUM"))

        # register constant bias APs used by scalar.activation float biases
        kt0 = wpool.tile([128, 1], F32, tag="konst_0")
        nc.vector.memset(kt0, 0.0)
        nc.const_aps.aps[(F32, 0.0)] = kt0[:]

        # preload ACT tables (Exp for softmax, Prelu for conv) with dummy ops
        # so the 1.5us table loads happen during the initial weight DMAs
        dumm = wpool.tile([128, 1], F32, tag="dummy")
        nc.scalar.activation(dumm, kt0, AF.Exp)
        nc.scalar.activation(dumm, kt0, AF.Prelu, alpha=kt0)
        nc.scalar.activation(dumm, kt0, AF.Relu)

        # ---------------- load weights (host-prepped layouts) -------------
        AT_sb = wpool.tile([128, 2, C], F16, tag="AT")
        nc.sync.dma_start(out=AT_sb, in_=AT_h[:])
        u2 = wpool.tile([128, 2, 2], F16, tag="u2")
        nc.sync.dma_start(out=u2, in_=u2_h[:])
        WgT = wpool.tile([128, 2, C], F16, tag="wgT")
        nc.sync.dma_start(out=WgT, in_=WgT_h[:])

        def load_vec(h, n, name):
            ch = max(n // 128, 1)
            p = min(n, 128)
            t = wpool.tile([p, ch], F32, tag=f"v_{name}")
            nc.sync.dma_start(out=t, in_=h[:].rearrange("(a p) -> p a", p=p))
            return t

        g_s = load_vec(g_s_h, C, "g_s")
        g_b = load_vec(g_b_h, C, "g_b")
        fi_s = load_vec(fi_s_h, C, "fi_s")
        fi_b = load_vec(fi_b_h, C, "fi_b")
        m_s = load_vec(m_s_h, 128, "m_s")
        m_b = load_vec(m_b_h, 128, "m_b")
        prelu_sb = wpool.tile([128, 1], F32, tag="prelu")
        nc.sync.dma_start(out=prelu_sb,
                          in_=prelu_h[:].unsqueeze(0).to_broadcast([128, 1]))

        # ---------------- hoisted zf stage (all BL batches at once) --------
        # zf is tiny (49 cols/batch): run azf/zf_g/zf_gT for all 4 batches in
        # one go at startup -> fewer small matmuls + fewer fixed-cost ACT ops
        zf_all = wpool.tile([128, 2, BL, NZ], F16, tag="zf_all")
        nc.sync.dma_start(out=zf_all, in_=zf_h[:])
        azf_all = wpool.tile([128, 2, BL, NZ], F16, tag="azf_all")
        zfg_all = wpool.tile([128, 2, BL, 128], BF16, tag="zfg_all")
        zfgT_all = wpool.tile([128, 2, BL, 128], BF16, tag="zfgT_all")

        def emit_zf_stage():
            ps_az = pps.tile([128, 2, BL * NZ], F32, tag="psmall")
            for ih in range(2):
                for jc in range(2):
                    nc.tensor.matmul(ps_az[:, ih, :],
                                     AT_sb[:, jc, ih * 128:(ih + 1) * 128],
                                     zf_all[:, jc], start=(jc == 0),
                                     stop=(jc == 1))
            nc.scalar.activation(azf_all, ps_az, AF.Copy)

            nc.vector.memset(zfg_all, 0.0)
            for a in range(2):
                ps_g = pps.tile([128, BL * NZ], F32, tag="psmall")
                for jc in range(2):
                    nc.tensor.matmul(ps_g, WgT[:, jc, a * 128:(a + 1) * 128],
                                     zf_all[:, jc], start=(jc == 0),
                                     stop=(jc == 1))
                nc.scalar.activation(
                    zfg_all[:, a, :, :NZ],
                    ps_g.rearrange("p (b j) -> p b j", j=NZ), AF.Relu,
                    bias=g_b[:, a:a + 1], scale=g_s[:, a:a + 1])
            nc.sync.dma_start_transpose(
                out=zfgT_all.rearrange("p a b j -> p (a b) j"),
                in_=zfg_all.rearrange("p a b j -> p (a b j)"))

        # ---------------- software-pipelined per-batch stages ----------------
        st = [dict() for _ in range(BL)]

        def emit_load(b):
            s = st[b]
            xf_r = dbl.tile([128, 2, NX], F16, tag="xf_r", name="xf_r")
            nc.sync.dma_start(
                out=xf_r,
                in_=xf_h[b].rearrange("(a p) h w -> p a (h w)", p=128))
            s['xf_r'] = xf_r

        def emit_front_p1(b):
            s = st[b]
            xf_r = s['xf_r']

            # term2[m] = u . zf[:, m]  -> exp bias row [49, 1]
            ps_t2 = pps.tile([NZ, 2], F32, tag="psmall")
            for jc in range(2):
                nc.tensor.matmul(ps_t2, zf_all[:, jc, b, :], u2[:, jc, :],
                                 start=(jc == 0), stop=(jc == 1))
            exp_bias = vpool.tile([NZ, 1], F32, tag="exp_bias", name="exp_bias")
            nc.vector.tensor_scalar(exp_bias, ps_t2[:, 0:1], EXP_SHIFT, None,
                                    ALU.add)

            # simT = azf^T xf  [49, 961] fp16
            ps_sT = pp.tile([128, 1024], F32, tag="mm961")
            for f0, fw in FSPLITS:
                sl = slice(f0, f0 + fw)
                for k in range(2):
                    nc.tensor.matmul(ps_sT[:NZ, sl], azf_all[:, k, b, :],
                                     xf_r[:, k, sl], start=(k == 0), stop=(k == 1))
            expT = apool.tile([NZ, NX], BF16, tag="expT")
            nc.scalar.activation(expT, ps_sT[:NZ, :NX], AF.Exp,
                                 bias=exp_bias)

            # -- g path fills the PE while exp cooks --
            xf_g = apool.tile([128, 2, NX], F16, tag="xf_g")
            for a in range(2):
                ps_g = pp.tile([128, 1024], F32, tag="mm961")
                for f0, fw in FSPLITS:
                    sl = slice(f0, f0 + fw)
                    for k in range(2):
                        nc.tensor.matmul(
                            ps_g[:, sl], WgT[:, k, a * 128:(a + 1) * 128],
                            xf_r[:, k, sl],
                            start=(k == 0), stop=(k == 1))
                nc.scalar.activation(xf_g[:, a, :], ps_g[:, :NX], AF.Relu,
                                     bias=g_b[:, a:a + 1], scale=g_s[:, a:a + 1])

            s['expT'] = expT
            s['xf_g'] = xf_g

        def emit_front_p2(b):
            s = st[b]
            expT, xf_g = s['expT'], s['xf_g']

            # den broadcast to all 128 partitions via ones[49,128] lhsT
            ps_den = pp.tile([128, 1024], F32, tag="mm961")
            for f0, fw in FSPLITS:
                sl = slice(f0, f0 + fw)
                nc.tensor.matmul(ps_den[:, sl], ones49, expT[:, sl],
                                 start=True, stop=True)
            # rdbc = 1/den on DVE (no Ln/Exp -> act table stays put)
            rdbc = apool.tile([128, NX], F32, tag="rdbc")
            nc.vector.reciprocal_approx_fast(out=rdbc, in_=ps_den[:, :NX])
            # normalized attention weights on GPSIMD (SBUF-only op)
            expT_s = apool.tile([NZ, NX], BF16, tag="expT_s")
            nc.gpsimd.tensor_mul(expT_s, expT, rdbc[:NZ, :])

            # emb = zf_g @ (expT/den)  [256, 961]
            emb = apool.tile([128, 2, NX], F16, tag="emb")
            for a in range(2):
                ps = pp.tile([128, 1024], F32, tag="mm961")
                for f0, fw in FSPLITS:
                    sl = slice(f0, f0 + fw)
                    nc.tensor.matmul(ps[:, sl],
                                     zfgT_all[:NZ, a, b, :],
                                     expT_s[:, sl], start=True, stop=True)
                nc.vector.tensor_copy(emb[:, a, :], ps[:, :NX])

            # gat: fi matmul -> relu(bn) -> fp16 contiguous gat_sb
            gat_sb = dbl.tile([128, 2, GSB_W], F16, tag="gat_sb")
            nc.vector.memset(gat_sb[:, :, NX:], 0.0)
            s['gat_sb'] = gat_sb
            gat_pad = gat_pads[b % 2]
            s['gat_pad'] = gat_pad
            for a in range(2):
                ps = pp.tile([128, 1024], F32, tag="mm961")
                for f0, fw in FSPLITS:
                    sl = slice(f0, f0 + fw)
                    for k in range(4):
                        rhs = emb[:, k, sl] if k < 2 else xf_g[:, k - 2, sl]
                        nc.tensor.matmul(ps[:, sl],
                                         WfiT[:, k, a * 128:(a + 1) * 128],
                                         rhs, start=(k == 0), stop=(k == 3))
                nc.scalar.activation(gat_sb[:, a, :NX], ps[:, :NX], AF.Relu,
                                     bias=fi_b[:, a:a + 1], scale=fi_s[:, a:a + 1])
                # padded copy for the conv windows (fp16, 2x DVE)
                nc.vector.tensor_copy(
                    gat_pad[:, a, 1:1 + HX, 1:1 + WX],
                    gat_sb[:, a, :NX].rearrange("p (h w) -> p h w", w=WX))

        def emit_conv(b):
            s = st[b]
            gat_pad = s['gat_pad']
            # e = prelu(bn(conv3x3(gat))) -> ef [128, 961] fp16
            # the two row-splits run in PE column groups h0/h64 concurrently
            ef = apool.tile([128, NX], F16, tag="ef")
            ps_e = pp.tile([128, 1024], F32, tag="mm961")
            first = {0: True, 64: True}
            for t in range(9):
                dy, dx = t // 3, t % 3
                for k in range(2):
                    for r0, nr, po in CSPLITS:
                        nc.tensor.matmul(
                            ps_e[po:po + CM, :nr * 32], WmT[:, k, t, :],
                            gat_pad[:, k, dy + r0:dy + r0 + nr, dx:dx + 32],
                            start=first[po], stop=(t == 8 and k == 1))
                        first[po] = False
            # e = prelu(m_s*conv + m_b) in one ACT op per split; each split's
            # cross-half replica DMA starts right after its own eviction
            for r0, nr, po in CSPLITS:
                pv = ps_e[po:po + CM, :nr * 32].rearrange(
                    "p (r w) -> p r w", w=32)[:, :, :WX]
                ov = slice(r0 * WX, (r0 + nr) * WX)
                nc.scalar.activation(
                    ef[po:po + CM, ov].rearrange("p (r w) -> p r w", w=WX), pv,
                    AF.Prelu, bias=m_b[po:po + CM], scale=m_s[po:po + CM],
                    alpha=prelu_sb[po:po + CM])
                # gpsimd-issued DMA: keeps the sync queue free for transposes
                if po == 0:
                    nc.gpsimd.dma_start(out=ef[CM:128, :CB], in_=ef[:CM, :CB])
                else:
                    nc.gpsimd.dma_start(out=ef[:CM, CB:NX], in_=ef[CM:128, CB:NX])
            s['ef'] = ef

            # gatT (n-major, fp16) via DMA XBAR transpose (off the PE);
            # all 16 128x128 blocks batched into ONE instruction
            gat_sb = s['gat_sb']
            gatT = dbl.tile([128, 2, 8, 128], F16, tag="gatT")
            s['gatT'] = gatT
            nc.sync.dma_start_transpose(
                out=gatT.rearrange("p a ci j -> p (a ci) j"),
                in_=gat_sb.rearrange("p a j -> p (a j)"))

        def emit_S(b, prange=(0, 4), last=False):
            s = st[b]
            ef = s['ef']
            # S = ef^T ef  [961, 961] fp16; chunk pairs run concurrently in
            # PE row-groups h0/h64 (K=64 each)
            if prange[0] == 0:
                s['S_sb'] = spool.tile([128, 8, NX], F16, tag="S", name="S_sb")
                s['v8'] = vpool.tile([128, 8, 8], F32, tag="v8", name="v8")
                s['tau'] = vpool.tile([128, 8], F32, tag="tau", name="tau_t")
                s['ntau'] = vpool.tile([128, 8], F32, tag="ntau", name="ntau_t")
            S_sb, v8 = s['S_sb'], s['v8']
            for p in range(*prange):
                c0, c1 = 2 * p, 2 * p + 1
                n00, cs0 = NCH[c0]
                n01, cs1 = NCH[c1]
                psA = pp.tile([128, 1024], F32, tag="mm961")
                psB = pp.tile([128, 1024], F32, tag="mm961")
                for f0, fw in FSPLITS:
                    sl = slice(f0, f0 + fw)
                    nc.tensor.matmul(psA[:cs0, sl], ef[:CM, n00:n00 + cs0],
                                     ef[:CM, sl], start=True, stop=True)
                    nc.tensor.matmul(psB[:cs1, sl],
                                     ef[CM:128, n01:n01 + cs1],
                                     ef[CM:128, sl], start=True, stop=True)
                if last:
                    # tail mode: top-8 straight from PSUM, evictions via ACT
                    for cc, css, pst in ((c0, cs0, psA), (c1, cs1, psB)):
                        nc.vector.max(out=v8[:css, cc, :], in_=pst[:css, :NX])
                    nc.scalar.activation(S_sb[:cs0, c0, :], psA[:cs0, :NX],
                                         AF.Copy)
                    nc.scalar.activation(S_sb[:cs1, c1, :], psB[:cs1, :NX],
                                         AF.Copy)
                else:
                    nc.vector.tensor_copy(S_sb[:cs0, c0, :], psA[:cs0, :NX])
                    nc.scalar.activation(S_sb[:cs1, c1, :], psB[:cs1, :NX],
                                         AF.Copy)
                    for cc, css in ((c0, cs0), (c1, cs1)):
                        nc.vector.max(out=v8[:css, cc, :],
                                      in_=S_sb[:css, cc, :])

        def emit_topk(b, chunks=slice(0, 8)):
            # cumsum + tau for a chunk range (v8 already sorted descending)
            s = st[b]
            nch = chunks.stop - chunks.start
            v8 = s['v8'][:, chunks, :]
            eng = nc.vector
            mA_t = vpool.tile([128, 8, 8], F32, tag="mA", name="mA_t")
            mB_t = vpool.tile([128, 8, 8], F32, tag="mB", name="mB_t")
            mA = mA_t[:, chunks, :]
            mB = mB_t[:, chunks, :]
            # Hillis-Steele cumsum over 8
            eng.tensor_tensor(mA[:, :, 1:8], v8[:, :, 1:8],
                              v8[:, :, 0:7], ALU.add)
            eng.tensor_copy(mA[:, :, 0:1], v8[:, :, 0:1])
            eng.tensor_tensor(mB[:, :, 2:8], mA[:, :, 2:8],
                              mA[:, :, 0:6], ALU.add)
            eng.tensor_copy(mB[:, :, 0:2], mA[:, :, 0:2])
            eng.tensor_tensor(mA[:, :, 4:8], mB[:, :, 4:8],
                              mB[:, :, 0:4], ALU.add)
            eng.tensor_copy(mA[:, :, 0:4], mB[:, :, 0:4])
            # t_j = (cumsum_j - 1) / j ; tau = max_j t_j
            nc.vector.scalar_tensor_tensor(mB, mA, 1.0,
                                            rj.to_broadcast([128, nch, 8]),
                                            op0=ALU.subtract, op1=ALU.mult)
            tau, ntau = s['tau'], s['ntau']
            eng.tensor_reduce(tau[:, chunks], mB, mybir.AxisListType.X, ALU.max)
            nc.vector.tensor_scalar(ntau[:, chunks], tau[:, chunks], -1.0,
                                    None, ALU.mult)

        def emit_P_chunk(s, ci, P_sb):
            S_sb, tau, ntau = s['S_sb'], s['tau'], s['ntau']
            n0, cs = NCH[ci]
            if ci % 2 == 0:
                nc.vector.tensor_scalar(P_sb[:cs, ci, :], S_sb[:cs, ci, :],
                                        tau[:cs, ci:ci + 1], 0.0,
                                        ALU.subtract, ALU.max)
            else:
                nc.scalar.activation(P_sb[:cs, ci, :], S_sb[:cs, ci, :],
                                     AF.Relu, bias=ntau[:cs, ci:ci + 1])
            # fold the "+gat" residual into P: out = gat @ (P + I)
            nc.vector.tensor_tensor(P_sb[:cs, ci, n0:n0 + cs],
                                    P_sb[:cs, ci, n0:n0 + cs],
                                    ident_h[:cs, :cs], ALU.add)

        def emit_P(b):
            # P = relu(S - tau) in fp16, split DVE/ACT; emitted late so the
            # ACT chunks don't head-of-line-block the next batch's copies
            s = st[b]
            P_sb = spool.tile([128, 8, NX], F16, tag="P")
            s['P_sb'] = P_sb
            for ci in range(8):
                emit_P_chunk(s, ci, P_sb)

        def emit_out(b):
            s = st[b]
            gatT, P_sb = s['gatT'], s['P_sb']
            out_sb = dbl.tile([128, 2, NX], F32, tag="out_sb")
            for a in range(2):
                ps = pp.tile([128, 1024], F32, tag="mm961")
                for f0, fw in FSPLITS:
                    sl = slice(f0, f0 + fw)
                    for ci, (n0, cs) in enumerate(NCH):
                        nc.tensor.matmul(ps[:, sl],
                                         gatT[:cs, a, ci, :],
                                         P_sb[:cs, ci, sl],
                                         start=(ci == 0), stop=(ci == 7))
                nc.scalar.activation(out_sb[:, a, :], ps[:, :NX], AF.Copy)
            nc.sync.dma_start(
                out=out_h[b].rearrange("(a p) h w -> p a (h w)", p=128),
                in_=out_sb)

        def emit_out_tail(b):
            # last batch: interleave P chunk production with the
            # PSUM-accumulating out matmuls so the PE starts immediately
            s = st[b]
            gatT = s['gatT']
            P_sb = spool.tile([128, 8, NX], F16, tag="P")
            s['P_sb'] = P_sb
            ps0 = pp.tile([128, 1024], F32, tag="mm961")
            ps1 = pp.tile([128, 1024], F32, tag="mm961")
            pss = (ps0, ps1)
            for ci, (n0, cs) in enumerate(NCH):
                emit_P_chunk(s, ci, P_sb)
                for a in range(2):
                    for f0, fw in FSPLITS:
                        sl = slice(f0, f0 + fw)
                        nc.tensor.matmul(pss[a][:, sl],
                                         gatT[:cs, a, ci, :],
                                         P_sb[:cs, ci, sl],
                                         start=(ci == 0), stop=(ci == 7))
            out_sb = dbl.tile([128, 2, NX], F32, tag="out_sb")
            for a in range(2):
                nc.scalar.activation(out_sb[:, a, :], pss[a][:, :NX], AF.Copy)
                nc.sync.dma_start(
                    out=out_h[b].rearrange("(a p) h w -> p a (h w)",
                                           p=128)[:, a:a + 1, :],
                    in_=out_sb[:, a:a + 1, :])

        emit_load(0)
        emit_zf_stage()
        emit_front_p1(0)

        # ones [49, 128] bf16: den broadcast matmul lhsT (M=128)
        ones49 = wpool.tile([NZ, 128], BF16, tag="ones49")
        nc.vector.memset(ones49, 1.0)

        # identity (fp16) for the sparsemax residual fold
        ident_f32 = wpool.tile([128, 128], F32, tag="ident_f32")
        make_identity(nc, ident_f32)
        ident_h = wpool.tile([128, 128], F16, tag="ident_h")
        nc.vector.tensor_copy(ident_h, ident_f32)

        # 1/j constants for the sparsemax threshold, j = 1..16
        rj = wpool.tile([128, 1, 8], F32, tag="rj")
        for j in range(8):
            nc.vector.memset(rj[:, :, j:j + 1], 1.0 / (j + 1))

        # persistent padded gat buffers (fp16), borders zeroed once
        gat_pads = []
        for par in range(2):
            gp = wpool.tile([128, 2, PW, PWW], F16, tag=f"gat_pad{par}")
            for a in range(2):
                nc.vector.memset(gp[:, a, 0, :], 0.0)
                nc.vector.memset(gp[:, a, PW - 1, :], 0.0)
                nc.vector.memset(gp[:, a, 1:PW - 1, 0:1], 0.0)
                nc.vector.memset(gp[:, a, 1:PW - 1, 1 + WX:PWW], 0.0)
            gat_pads.append(gp)
        # bulk weights arrive after the small/startup-critical loads
        WfiT = wpool.tile([128, 4, C], F16, tag="wfiT")
        nc.sync.dma_start(out=WfiT, in_=WfiT_h[:])
        WmT = wpool.tile([128, 2, 9, CM], F16, tag="wmT")
        nc.sync.dma_start(out=WmT, in_=WmT_h[:])

        emit_front_p2(0)
        emit_conv(0)
        emit_S(0)
        for b in range(1, BL):
            last = (b == BL - 1)
            emit_load(b)
            emit_topk(b - 1)
            emit_front_p1(b)
            emit_front_p2(b)
            emit_conv(b)
            emit_P(b - 1)
            emit_out(b - 1)
            if last:
                emit_S(b, (0, 2), last=True)
                emit_topk(b, slice(0, 4))
                emit_S(b, (2, 4), last=True)
                emit_topk(b, slice(4, 8))
            else:
                emit_S(b)
        emit_out_tail(BL - 1)

    nc.compile()
    return nc


_CACHED = None


def _get_nc():
    global _CACHED
    if _CACHED is None:
        _CACHED = build_bass()
    return _CACHED


def make_in_maps(inputs):
    """Host-side layout prep + BN folds + batch sharding."""
    full = {k: np.ascontiguousarray(np.asarray(v, dtype=np.float32))
            for k, v in inputs.items()}
    w = {}
    # A = Wq^T Ws fold: azf = A @ zf via lhsT AT[j, i] = sum_o Ws[o,j] Wq[o,i]
    AT = full['Ws'].T @ full['Wq']                       # [j, i] = [256, 256]
    w['AT_nat'] = np.ascontiguousarray(
        AT.reshape(2, 128, C).transpose(1, 0, 2)).astype(np.float16)
    u = full['Ws'].T @ full['bq']                        # [256]
    u_r = u.reshape(2, 128).T                            # [128, 2]
    w['u2'] = np.ascontiguousarray(
        np.repeat(u_r[:, :, None], 2, axis=2)).astype(np.float16)
    w['WgT'] = np.ascontiguousarray(
        full['Wg'].reshape(C, 2, 128).transpose(2, 1, 0)).astype(np.float16)
    w['WfiT'] = np.ascontiguousarray(
        full['Wfi'].reshape(C, 4, 128).transpose(2, 1, 0)).astype(np.float16)
    w['WmT'] = np.ascontiguousarray(
        full['Wm'].reshape(CM, 2, 128, 9).transpose(2, 1, 3, 0)).astype(np.float16)

    # BN folds: y = s*x_conv + b, with s = gamma/sqrt(var+eps),
    # b = beta - mean*s + s*conv_bias
    def fold(gamma, beta, mean, var, conv_b):
        s = gamma / np.sqrt(var + EPS)
        return s.astype(np.float32), (beta - mean * s + s * conv_b).astype(np.float32)

    w['g_s'], w['g_b'] = fold(full['g_gamma'], full['g_beta'],
                              full['g_mean'], full['g_var'], full['bg'])
    w['fi_s'], w['fi_b'] = fold(full['fi_gamma'], full['fi_beta'],
                                full['fi_mean'], full['fi_var'], full['bfi'])
    m_s, m_b = fold(full['m_gamma'], full['m_beta'],
                    full['m_mean'], full['m_var'], full['bm'])
    w['m_s'] = np.tile(m_s, 2)                            # both psum halves
    w['m_b'] = np.tile(m_b, 2)
    w['prelu_a'] = full['prelu_a'].reshape(1)
    zf16 = full['zf'].astype(np.float16)
    xf16 = full['xf'].astype(np.float16)
    in_maps = []
    for c in range(NCORES):
        m = dict(w)
        zc = zf16[c * BL:(c + 1) * BL]          # [BL, C, HZ, WZ]
        m['zf'] = np.ascontiguousarray(
            zc.reshape(BL, 2, 128, NZ).transpose(2, 1, 0, 3))
        m['xf'] = xf16[c * BL:(c + 1) * BL]
        in_maps.append(m)
    return in_maps


def kernel(**inputs):
    nc = _get_nc()
    in_maps = make_in_maps(inputs)
    res = run_bass_kernel_spmd(nc, in_maps, core_ids=list(range(NCORES)))
    out = np.concatenate([r['out'] for r in res.results], axis=0)
    return out.astype(np.float32)


if __name__ == "__main__":
    # smoke-build
    nc = build_bass()
    print("built ok:",
          sum(len(b.instructions) for f in nc.m.functions for b in f.blocks),
          "instructions")


# revision 22
# speedup vs baseline: 2.3763x; 1.0980x over previous
"""Trainium2 Bass kernel for the GAT+HSPA cascade (nn_GAT_HSPA_Cascade).

Takes FULL inputs (B=32), shards batch across 8 NeuronCores (4 per core,
weights replicated), runs one SPMD Bass/Tile program, gathers full output.

v2 changes vs baseline (214us):
  - all BN folds / A = Wq^T Ws / u = Ws^T bq computed on HOST (numpy): kills
    the 13us Ln/Exp ACT-table thrash at startup
  - fp16 matmuls everywhere (PE runs 16-bit at ~0.9ns/row vs f32r 1.4);
    expT stays bf16 (exp range ~e^20 overflows fp16)
  - conv3x3 row-splits paired into PE column groups h0/h64 (concurrent
    64-col matmuls) -> conv time ~halves
  - gatT + zf_gT transposes moved off the PE onto the DMA XBAR
    (dma_start_transpose), killing 16 PE transposes + their PSUM evictions
  - S_sb/P in fp16 (2x DVE on max8/P production; numpy-verified 1.4e-3)
  - ACT tables (Exp/Prelu) preloaded via dummy ops during initial DMAs

Per-batch pipeline on each core (b = one image):
  azf  = (Ws^T Wq)^T zf            [256, 49]
  simT = azf^T xf                  [49, 961]
  expT = exp(simT + bq^T zf_t + SHIFT)   (bf16)
  den  = ones^T expT ; rdbc = 1/den on DVE
  xf_g = relu(bn(Wg@xf)) ; zf_g likewise
  emb  = (zf_g @ expT) * rdbc
  gat  = relu(bn(Wfi@[emb;xf_g]))  (fp16)
  gatT via DMA XBAR transpose
  e    = prelu(bn(conv3x3(gat)))   [64, 961] col-group-paired
  S    = e^T e                     [961, 961] row-group-paired fp16
  P    = sparsemax_row(S) via top-16 (max8 even/odd + bitonic merge +
         tau = max_j (cumsum_j - 1)/j), exact for support <= 15
  out  = gat @ (P + I)
"""

import numpy as np

import concourse.bass as bass
import concourse.mybir as mybir
from concourse import bacc
from concourse.tile import TileContext
from concourse.masks import make_identity
from concourse.bass_utils import run_bass_kernel_spmd

F32 = mybir.dt.float32
F16 = mybir.dt.float16
BF16 = mybir.dt.bfloat16
AF = mybir.ActivationFunctionType
ALU = mybir.AluOpType

# problem constants (hardcoded per contract)
B, C, CM = 32, 256, 64
HX, WX, HZ, WZ = 31, 31, 7, 7
NX, NZ = HX * WX, HZ * WZ          # 961, 49
NCORES = 8
BL = B // NCORES                   # 4 batches per core
EPS = 1e-5
PW = HX + 2                        # padded spatial height 33
EXP_SHIFT = -40.0                  # constant softmax shift (sim |max| ~ 60)

# flat splits of the 961-wide free dim, 512-aligned for PSUM banks
FSPLITS = [(0, 512), (512, 449)]
# conv row-splits -> (row0, nrows, psum/ef partition offset); col groups
CSPLITS = [(0, 16, 0), (16, 15, 64)]
CB = 16 * WX                       # 496: col boundary between the splits
PWW = 34  # padded gat buffer row width (31 + 1 left + 2 right)
# 961 rows -> 8 partition chunks
NCH = [(ci * 128, min(128, NX - ci * 128)) for ci in range(8)]
GSB_W = 1024  # gat_sb padded width


def build_bass():
    nc = bacc.Bacc(None, target_bir_lowering=False)

    # ---------------- DRAM I/O ----------------
    # weights arrive in kernel-friendly layouts, BN already folded (host)
    zf_h = nc.dram_tensor("zf", [128, 2, BL, NZ], F16, kind="ExternalInput")
    xf_h = nc.dram_tensor("xf", [BL, C, HX, WX], F16, kind="ExternalInput")
    AT_h = nc.dram_tensor("AT_nat", [128, 2, C], F16, kind="ExternalInput")
    u2_h = nc.dram_tensor("u2", [128, 2, 2], F16, kind="ExternalInput")
    WgT_h = nc.dram_tensor("WgT", [128, 2, C], F16, kind="ExternalInput")
    WfiT_h = nc.dram_tensor("WfiT", [128, 4, C], F16, kind="ExternalInput")
    WmT_h = nc.dram_tensor("WmT", [128, 2, 9, CM], F16, kind="ExternalInput")
    g_s_h = nc.dram_tensor("g_s", [C], F32, kind="ExternalInput")
    g_b_h = nc.dram_tensor("g_b", [C], F32, kind="ExternalInput")
    fi_s_h = nc.dram_tensor("fi_s", [C], F32, kind="ExternalInput")
    fi_b_h = nc.dram_tensor("fi_b", [C], F32, kind="ExternalInput")
    m_s_h = nc.dram_tensor("m_s", [128], F32, kind="ExternalInput")
    m_b_h = nc.dram_tensor("m_b", [128], F32, kind="ExternalInput")
    prelu_h = nc.dram_tensor("prelu_a", [1], F32, kind="ExternalInput")
    out_h = nc.dram_tensor("out", [BL, C, HX, WX], F32, kind="ExternalOutput")

    from contextlib import ExitStack
    with TileContext(nc) as tc, ExitStack() as ctx:
        wpool = ctx.enter_context(tc.tile_pool(name="weights", bufs=1))
        apool = ctx.enter_context(tc.tile_pool(name="acts", bufs=1))
        dbl = ctx.enter_context(tc.tile_pool(name="dbl", bufs=2))
        spool = ctx.enter_context(tc.tile_pool(name="smat", bufs=1))
        vpool = ctx.enter_context(tc.tile_pool(name="vsmall", bufs=2))
        pp = ctx.enter_context(tc.tile_pool(name="ps_big", bufs=3, space="PSUM"))
        pps = ctx.enter_context(tc.tile_pool(name="ps_small", bufs=2, space="PSUM"))

        # register constant bias APs used by scalar.activation float biases
        kt0 = wpool.tile([128, 1], F32, tag="konst_0")
        nc.vector.memset(kt0, 0.0)
        nc.const_aps.aps[(F32, 0.0)] = kt0[:]

        # preload ACT tables (Exp for softmax, Prelu for conv) with dummy ops
        # so the 1.5us table loads happen during the initial weight DMAs
        dumm = wpool.tile([128, 1], F32, tag="dummy")
        nc.scalar.activation(dumm, kt0, AF.Exp)
        nc.scalar.activation(dumm, kt0, AF.Prelu, alpha=kt0)
        nc.scalar.activation(dumm, kt0, AF.Relu)

        # ---------------- load weights (host-prepped layouts) -------------
        AT_sb = wpool.tile([128, 2, C], F16, tag="AT")
        nc.sync.dma_start(out=AT_sb, in_=AT_h[:])
        u2 = wpool.tile([128, 2, 2], F16, tag="u2")
        nc.sync.dma_start(out=u2, in_=u2_h[:])
        WgT = wpool.tile([128, 2, C], F16, tag="wgT")
        nc.sync.dma_start(out=WgT, in_=WgT_h[:])

        def load_vec(h, n, name):
            ch = max(n // 128, 1)
            p = min(n, 128)
            t = wpool.tile([p, ch], F32, tag=f"v_{name}")
            nc.sync.dma_start(out=t, in_=h[:].rearrange("(a p) -> p a", p=p))
            return t

        g_s = load_vec(g_s_h, C, "g_s")
        g_b = load_vec(g_b_h, C, "g_b")
        fi_s = load_vec(fi_s_h, C, "fi_s")
        fi_b = load_vec(fi_b_h, C, "fi_b")
        m_s = load_vec(m_s_h, 128, "m_s")
        m_b = load_vec(m_b_h, 128, "m_b")
        prelu_sb = wpool.tile([128, 1], F32, tag="prelu")
        nc.sync.dma_start(out=prelu_sb,
                          in_=prelu_h[:].unsqueeze(0).to_broadcast([128, 1]))

        # ---------------- hoisted zf stage (all BL batches at once) --------
        # zf is tiny (49 cols/batch): run azf/zf_g/zf_gT for all 4 batches in
        # one go at startup -> fewer small matmuls + fewer fixed-cost ACT ops
        zf_all = wpool.tile([128, 2, BL, NZ], F16, tag="zf_all")
        nc.sync.dma_start(out=zf_all, in_=zf_h[:])
        azf_all = wpool.tile([128, 2, BL, NZ], F16, tag="azf_all")
        zfg_all = wpool.tile([128, 2, BL, 128], BF16, tag="zfg_all")
        zfgT_all = wpool.tile([128, 2, BL, 128], BF16, tag="zfgT_all")

        def emit_zf_stage():
            ps_az = pps.tile([128, 2, BL * NZ], F32, tag="psmall")
            for ih in range(2):
                for jc in range(2):
                    nc.tensor.matmul(ps_az[:, ih, :],
                                     AT_sb[:, jc, ih * 128:(ih + 1) * 128],
                                     zf_all[:, jc], start=(jc == 0),
                                     stop=(jc == 1))
            nc.scalar.activation(azf_all, ps_az, AF.Copy)

            nc.vector.memset(zfg_all, 0.0)
            for a in range(2):
                ps_g = pps.tile([128, BL * NZ], F32, tag="psmall")
                for jc in range(2):
                    nc.tensor.matmul(ps_g, WgT[:, jc, a * 128:(a + 1) * 128],
                                     zf_all[:, jc], start=(jc == 0),
                                     stop=(jc == 1))
                nc.scalar.activation(
                    zfg_all[:, a, :, :NZ],
                    ps_g.rearrange("p (b j) -> p b j", j=NZ), AF.Relu,
                    bias=g_b[:, a:a + 1], scale=g_s[:, a:a + 1])
            nc.sync.dma_start_transpose(
                out=zfgT_all.rearrange("p a b j -> p (a b) j"),
                in_=zfg_all.rearrange("p a b j -> p (a b j)"))

        # ---------------- software-pipelined per-batch stages ----------------
        st = [dict() for _ in range(BL)]

        def emit_load(b):
            s = st[b]
            xf_r = dbl.tile([128, 2, NX], F16, tag="xf_r", name="xf_r")
            nc.sync.dma_start(
                out=xf_r,
                in_=xf_h[b].rearrange("(a p) h w -> p a (h w)", p=128))
            s['xf_r'] = xf_r

        def emit_front_p1(b):
            s = st[b]
            xf_r = s['xf_r']

            # term2[m] = u . zf[:, m]  -> exp bias row [49, 1]
            ps_t2 = pps.tile([NZ, 2], F32, tag="psmall")
            for jc in range(2):
                nc.tensor.matmul(ps_t2, zf_all[:, jc, b, :], u2[:, jc, :],
                                 start=(jc == 0), stop=(jc == 1))
            exp_bias = vpool.tile([NZ, 1], F32, tag="exp_bias", name="exp_bias")
            nc.vector.tensor_scalar(exp_bias, ps_t2[:, 0:1], EXP_SHIFT, None,
                                    ALU.add)

            # simT = azf^T xf  [49, 961] fp16
            ps_sT = pp.tile([128, 1024], F32, tag="mm961")
            for f0, fw in FSPLITS:
                sl = slice(f0, f0 + fw)
                for k in range(2):
                    nc.tensor.matmul(ps_sT[:NZ, sl], azf_all[:, k, b, :],
                                     xf_r[:, k, sl], start=(k == 0), stop=(k == 1))
            expT = apool.tile([NZ, NX], BF16, tag="expT")
            nc.scalar.activation(expT, ps_sT[:NZ, :NX], AF.Exp,
                                 bias=exp_bias)

            # -- g path fills the PE while exp cooks --
            xf_g = apool.tile([128, 2, NX], F16, tag="xf_g")
            for a in range(2):
                ps_g = pp.tile([128, 1024], F32, tag="mm961")
                for f0, fw in FSPLITS:
                    sl = slice(f0, f0 + fw)
                    for k in range(2):
                        nc.tensor.matmul(
                            ps_g[:, sl], WgT[:, k, a * 128:(a + 1) * 128],
                            xf_r[:, k, sl],
                            start=(k == 0), stop=(k == 1))
                nc.scalar.activation(xf_g[:, a, :], ps_g[:, :NX], AF.Relu,
                                     bias=g_b[:, a:a + 1], scale=g_s[:, a:a + 1])

            s['expT'] = expT
            s['xf_g'] = xf_g

        def emit_front_p2(b):
            s = st[b]
            expT, xf_g = s['expT'], s['xf_g']

            # den broadcast to all 128 partitions via ones[49,128] lhsT
            ps_den = pp.tile([128, 1024], F32, tag="mm961")
            for f0, fw in FSPLITS:
                sl = slice(f0, f0 + fw)
                nc.tensor.matmul(ps_den[:, sl], ones49, expT[:, sl],
                                 start=True, stop=True)
            # rdbc = 1/den on DVE (no Ln/Exp -> act table stays put)
            rdbc = apool.tile([128, NX], F32, tag="rdbc")
            nc.vector.reciprocal_approx_fast(out=rdbc, in_=ps_den[:, :NX])

            # emb = (zf_g @ expT) * rdbc  [256, 961]
            emb = apool.tile([128, 2, NX], F16, tag="emb")
            for a in range(2):
                ps = pp.tile([128, 1024], F32, tag="mm961")
                for f0, fw in FSPLITS:
                    sl = slice(f0, f0 + fw)
                    nc.tensor.matmul(ps[:, sl],
                                     zfgT_all[:NZ, a, b, :],
                                     expT[:, sl], start=True, stop=True)
                nc.vector.tensor_mul(emb[:, a, :], ps[:, :NX], rdbc)

            # gat: fi matmul -> relu(bn) -> fp16 contiguous gat_sb
            gat_sb = dbl.tile([128, 2, GSB_W], F16, tag="gat_sb")
            nc.vector.memset(gat_sb[:, :, NX:], 0.0)
            s['gat_sb'] = gat_sb
            gat_pad = gat_pads[b % 2]
            s['gat_pad'] = gat_pad
            for a in range(2):
                ps = pp.tile([128, 1024], F32, tag="mm961")
                for f0, fw in FSPLITS:
                    sl = slice(f0, f0 + fw)
                    for k in range(4):
                        rhs = emb[:, k, sl] if k < 2 else xf_g[:, k - 2, sl]
                        nc.tensor.matmul(ps[:, sl],
                                         WfiT[:, k, a * 128:(a + 1) * 128],
                                         rhs, start=(k == 0), stop=(k == 3))
                nc.scalar.activation(gat_sb[:, a, :NX], ps[:, :NX], AF.Relu,
                                     bias=fi_b[:, a:a + 1], scale=fi_s[:, a:a + 1])
                # padded copy for the conv windows (fp16, 2x DVE)
                nc.vector.tensor_copy(
                    gat_pad[:, a, 1:1 + HX, 1:1 + WX],
                    gat_sb[:, a, :NX].rearrange("p (h w) -> p h w", w=WX))

        def emit_conv(b):
            s = st[b]
            gat_pad = s['gat_pad']
            # e = prelu(bn(conv3x3(gat))) -> ef [128, 961] fp16
            # the two row-splits run in PE column groups h0/h64 concurrently
            ef = apool.tile([128, NX], F16, tag="ef")
            ps_e = pp.tile([128, 1024], F32, tag="mm961")
            first = {0: True, 64: True}
            for t in range(9):
                dy, dx = t // 3, t % 3
                for k in range(2):
                    for r0, nr, po in CSPLITS:
                        nc.tensor.matmul(
                            ps_e[po:po + CM, :nr * 32], WmT[:, k, t, :],
                            gat_pad[:, k, dy + r0:dy + r0 + nr, dx:dx + 32],
                            start=first[po], stop=(t == 8 and k == 1))
                        first[po] = False
            # e = prelu(m_s*conv + m_b) in one ACT op per split; each split's
            # cross-half replica DMA starts right after its own eviction
            for r0, nr, po in CSPLITS:
                pv = ps_e[po:po + CM, :nr * 32].rearrange(
                    "p (r w) -> p r w", w=32)[:, :, :WX]
                ov = slice(r0 * WX, (r0 + nr) * WX)
                nc.scalar.activation(
                    ef[po:po + CM, ov].rearrange("p (r w) -> p r w", w=WX), pv,
                    AF.Prelu, bias=m_b[po:po + CM], scale=m_s[po:po + CM],
                    alpha=prelu_sb[po:po + CM])
                # gpsimd-issued DMA: keeps the sync queue free for transposes
                if po == 0:
                    nc.gpsimd.dma_start(out=ef[CM:128, :CB], in_=ef[:CM, :CB])
                else:
                    nc.gpsimd.dma_start(out=ef[:CM, CB:NX], in_=ef[CM:128, CB:NX])
            s['ef'] = ef

            # gatT (n-major, fp16) via DMA XBAR transpose (off the PE);
            # all 16 128x128 blocks batched into ONE instruction
            gat_sb = s['gat_sb']
            gatT = dbl.tile([128, 2, 8, 128], F16, tag="gatT")
            s['gatT'] = gatT
            nc.sync.dma_start_transpose(
                out=gatT.rearrange("p a ci j -> p (a ci) j"),
                in_=gat_sb.rearrange("p a j -> p (a j)"))

        def emit_S(b, prange=(0, 4), last=False):
            s = st[b]
            ef = s['ef']
            # S = ef^T ef  [961, 961] fp16; chunk pairs run concurrently in
            # PE row-groups h0/h64 (K=64 each)
            if prange[0] == 0:
                s['S_sb'] = spool.tile([128, 8, NX], F16, tag="S", name="S_sb")
                s['v8'] = vpool.tile([128, 8, 8], F32, tag="v8", name="v8")
                s['tau'] = vpool.tile([128, 8], F32, tag="tau", name="tau_t")
                s['ntau'] = vpool.tile([128, 8], F32, tag="ntau", name="ntau_t")
            S_sb, v8 = s['S_sb'], s['v8']
            for p in range(*prange):
                c0, c1 = 2 * p, 2 * p + 1
                n00, cs0 = NCH[c0]
                n01, cs1 = NCH[c1]
                psA = pp.tile([128, 1024], F32, tag="mm961")
                psB = pp.tile([128, 1024], F32, tag="mm961")
                for f0, fw in FSPLITS:
                    sl = slice(f0, f0 + fw)
                    nc.tensor.matmul(psA[:cs0, sl], ef[:CM, n00:n00 + cs0],
                                     ef[:CM, sl], start=True, stop=True)
                    nc.tensor.matmul(psB[:cs1, sl],
                                     ef[CM:128, n01:n01 + cs1],
                                     ef[CM:128, sl], start=True, stop=True)
                if last:
                    # tail mode: top-8 straight from PSUM, evictions via ACT
                    for cc, css, pst in ((c0, cs0, psA), (c1, cs1, psB)):
                        nc.vector.max(out=v8[:css, cc, :], in_=pst[:css, :NX])
                    nc.scalar.activation(S_sb[:cs0, c0, :], psA[:cs0, :NX],
                                         AF.Copy)
                    nc.scalar.activation(S_sb[:cs1, c1, :], psB[:cs1, :NX],
                                         AF.Copy)
                else:
                    nc.vector.tensor_copy(S_sb[:cs0, c0, :], psA[:cs0, :NX])
                    nc.scalar.activation(S_sb[:cs1, c1, :], psB[:cs1, :NX],
                                         AF.Copy)
                    for cc, css in ((c0, cs0), (c1, cs1)):
                        nc.vector.max(out=v8[:css, cc, :],
                                      in_=S_sb[:css, cc, :])

        def emit_topk(b, chunks=slice(0, 8)):
            # cumsum + tau for a chunk range (v8 already sorted descending)
            s = st[b]
            nch = chunks.stop - chunks.start
            v8 = s['v8'][:, chunks, :]
            eng = nc.vector
            mA_t = vpool.tile([128, 8, 8], F32, tag="mA", name="mA_t")
            mB_t = vpool.tile([128, 8, 8], F32, tag="mB", name="mB_t")
            mA = mA_t[:, chunks, :]
            mB = mB_t[:, chunks, :]
            # Hillis-Steele cumsum over 8
            eng.tensor_tensor(mA[:, :, 1:8], v8[:, :, 1:8],
                              v8[:, :, 0:7], ALU.add)
            eng.tensor_copy(mA[:, :, 0:1], v8[:, :, 0:1])
            eng.tensor_tensor(mB[:, :, 2:8], mA[:, :, 2:8],
                              mA[:, :, 0:6], ALU.add)
            eng.tensor_copy(mB[:, :, 0:2], mA[:, :, 0:2])
            eng.tensor_tensor(mA[:, :, 4:8], mB[:, :, 4:8],
                              mB[:, :, 0:4], ALU.add)
            eng.tensor_copy(mA[:, :, 0:4], mB[:, :, 0:4])
            # t_j = (cumsum_j - 1) / j ; tau = max_j t_j
            nc.vector.scalar_tensor_tensor(mB, mA, 1.0,
                                            rj.to_broadcast([128, nch, 8]),
                                            op0=ALU.subtract, op1=ALU.mult)
            tau, ntau = s['tau'], s['ntau']
            eng.tensor_reduce(tau[:, chunks], mB, mybir.AxisListType.X, ALU.max)
            nc.vector.tensor_scalar(ntau[:, chunks], tau[:, chunks], -1.0,
                                    None, ALU.mult)

        def emit_P_chunk(s, ci, P_sb):
            S_sb, tau, ntau = s['S_sb'], s['tau'], s['ntau']
            n0, cs = NCH[ci]
            if ci % 2 == 0:
                nc.vector.tensor_scalar(P_sb[:cs, ci, :], S_sb[:cs, ci, :],
                                        tau[:cs, ci:ci + 1], 0.0,
                                        ALU.subtract, ALU.max)
            else:
                nc.scalar.activation(P_sb[:cs, ci, :], S_sb[:cs, ci, :],
                                     AF.Relu, bias=ntau[:cs, ci:ci + 1])
            # fold the "+gat" residual into P: out = gat @ (P + I)
            nc.vector.tensor_tensor(P_sb[:cs, ci, n0:n0 + cs],
                                    P_sb[:cs, ci, n0:n0 + cs],
                                    ident_h[:cs, :cs], ALU.add)

        def emit_P(b):
            # P = relu(S - tau) in fp16, split DVE/ACT; emitted late so the
            # ACT chunks don't head-of-line-block the next batch's copies
            s = st[b]
            P_sb = spool.tile([128, 8, NX], F16, tag="P")
            s['P_sb'] = P_sb
            for ci in range(8):
                emit_P_chunk(s, ci, P_sb)

        def emit_out(b):
            s = st[b]
            gatT, P_sb = s['gatT'], s['P_sb']
            out_sb = dbl.tile([128, 2, NX], F32, tag="out_sb")
            for a in range(2):
                ps = pp.tile([128, 1024], F32, tag="mm961")
                for f0, fw in FSPLITS:
                    sl = slice(f0, f0 + fw)
                    for ci, (n0, cs) in enumerate(NCH):
                        nc.tensor.matmul(ps[:, sl],
                                         gatT[:cs, a, ci, :],
                                         P_sb[:cs, ci, sl],
                                         start=(ci == 0), stop=(ci == 7))
                nc.scalar.activation(out_sb[:, a, :], ps[:, :NX], AF.Copy)
            nc.sync.dma_start(
                out=out_h[b].rearrange("(a p) h w -> p a (h w)", p=128),
                in_=out_sb)

        def emit_out_tail(b):
            # last batch: interleave P chunk production with the
            # PSUM-accumulating out matmuls so the PE starts immediately
            s = st[b]
            gatT = s['gatT']
            P_sb = spool.tile([128, 8, NX], F16, tag="P")
            s['P_sb'] = P_sb
            ps0 = pp.tile([128, 1024], F32, tag="mm961")
            ps1 = pp.tile([128, 1024], F32, tag="mm961")
            pss = (ps0, ps1)
            for ci, (n0, cs) in enumerate(NCH):
                emit_P_chunk(s, ci, P_sb)
                for a in range(2):
                    for f0, fw in FSPLITS:
                        sl = slice(f0, f0 + fw)
                        nc.tensor.matmul(pss[a][:, sl],
                                         gatT[:cs, a, ci, :],
                                         P_sb[:cs, ci, sl],
                                         start=(ci == 0), stop=(ci == 7))
            out_sb = dbl.tile([128, 2, NX], F32, tag="out_sb")
            for a in range(2):
                nc.scalar.activation(out_sb[:, a, :], pss[a][:, :NX], AF.Copy)
                nc.sync.dma_start(
                    out=out_h[b].rearrange("(a p) h w -> p a (h w)",
                                           p=128)[:, a:a + 1, :],
                    in_=out_sb[:, a:a + 1, :])

        emit_load(0)
        emit_zf_stage()
        emit_front_p1(0)

        # ones [49, 128] bf16: den broadcast matmul lhsT (M=128)
        ones49 = wpool.tile([NZ, 128], BF16, tag="ones49")
        nc.vector.memset(ones49, 1.0)

        # identity (fp16) for the sparsemax residual fold
        ident_f32 = wpool.tile([128, 128], F32, tag="ident_f32")
        make_identity(nc, ident_f32)
        ident_h = wpool.tile([128, 128], F16, tag="ident_h")
        nc.vector.tensor_copy(ident_h, ident_f32)

        # 1/j constants for the sparsemax threshold, j = 1..16
        rj = wpool.tile([128, 1, 8], F32, tag="rj")
        for j in range(8):
            nc.vector.memset(rj[:, :, j:j + 1], 1.0 / (j + 1))

        # persistent padded gat buffers (fp16), borders zeroed once
        gat_pads = []
        for par in range(2):
            gp = wpool.tile([128, 2, PW, PWW], F16, tag=f"gat_pad{par}")
            for a in range(2):
                nc.vector.memset(gp[:, a, 0, :], 0.0)
                nc.vector.memset(gp[:, a, PW - 1, :], 0.0)
                nc.vector.memset(gp[:, a, 1:PW - 1, 0:1], 0.0)
                nc.vector.memset(gp[:, a, 1:PW - 1, 1 + WX:PWW], 0.0)
            gat_pads.append(gp)
        # bulk weights arrive after the small/startup-critical loads
        WfiT = wpool.tile([128, 4, C], F16, tag="wfiT")
        nc.sync.dma_start(out=WfiT, in_=WfiT_h[:])
        WmT = wpool.tile([128, 2, 9, CM], F16, tag="wmT")
        nc.sync.dma_start(out=WmT, in_=WmT_h[:])

        emit_front_p2(0)
        emit_conv(0)
        emit_S(0)
        for b in range(1, BL):
            last = (b == BL - 1)
            emit_load(b)
            emit_topk(b - 1)
            emit_front_p1(b)
            emit_front_p2(b)
            emit_conv(b)
            emit_P(b - 1)
            emit_out(b - 1)
            if last:
                emit_S(b, (0, 2), last=True)
                emit_topk(b, slice(0, 4))
                emit_S(b, (2, 4), last=True)
                emit_topk(b, slice(4, 8))
            else:
                emit_S(b)
        emit_out_tail(BL - 1)

    nc.compile()
    return nc


_CACHED = None


def _get_nc():
    global _CACHED
    if _CACHED is None:
        _CACHED = build_bass()
    return _CACHED


def make_in_maps(inputs):
    """Host-side layout prep + BN folds + batch sharding."""
    full = {k: np.ascontiguousarray(np.asarray(v, dtype=np.float32))
            for k, v in inputs.items()}
    w = {}
    # A = Wq^T Ws fold: azf = A @ zf via lhsT AT[j, i] = sum_o Ws[o,j] Wq[o,i]
    AT = full['Ws'].T @ full['Wq']                       # [j, i] = [256, 256]
    w['AT_nat'] = np.ascontiguousarray(
        AT.reshape(2, 128, C).transpose(1, 0, 2)).astype(np.float16)
    u = full['Ws'].T @ full['bq']                        # [256]
    u_r = u.reshape(2, 128).T                            # [128, 2]
    w['u2'] = np.ascontiguousarray(
        np.repeat(u_r[:, :, None], 2, axis=2)).astype(np.float16)
    w['WgT'] = np.ascontiguousarray(
        full['Wg'].reshape(C, 2, 128).transpose(2, 1, 0)).astype(np.float16)
    w['WfiT'] = np.ascontiguousarray(
        full['Wfi'].reshape(C, 4, 128).transpose(2, 1, 0)).astype(np.float16)
    w['WmT'] = np.ascontiguousarray(
        full['Wm'].reshape(CM, 2, 128, 9).transpose(2, 1, 3, 0)).astype(np.float16)

    # BN folds: y = s*x_conv + b, with s = gamma/sqrt(var+eps),
    # b = beta - mean*s + s*conv_bias
    def fold(gamma, beta, mean, var, conv_b):
        s = gamma / np.sqrt(var + EPS)
        return s.astype(np.float32), (beta - mean * s + s * conv_b).astype(np.float32)

    w['g_s'], w['g_b'] = fold(full['g_gamma'], full['g_beta'],
                              full['g_mean'], full['g_var'], full['bg'])
    w['fi_s'], w['fi_b'] = fold(full['fi_gamma'], full['fi_beta'],
                                full['fi_mean'], full['fi_var'], full['bfi'])
    m_s, m_b = fold(full['m_gamma'], full['m_beta'],
                    full['m_mean'], full['m_var'], full['bm'])
    w['m_s'] = np.tile(m_s, 2)                            # both psum halves
    w['m_b'] = np.tile(m_b, 2)
    w['prelu_a'] = full['prelu_a'].reshape(1)
    zf16 = full['zf'].astype(np.float16)
    xf16 = full['xf'].astype(np.float16)
    in_maps = []
    for c in range(NCORES):
        m = dict(w)
        zc = zf16[c * BL:(c + 1) * BL]          # [BL, C, HZ, WZ]
        m['zf'] = np.ascontiguousarray(
            zc.reshape(BL, 2, 128, NZ).transpose(2, 1, 0, 3))
        m['xf'] = xf16[c * BL:(c + 1) * BL]
        in_maps.append(m)
    return in_maps


def kernel(**inputs):
    nc = _get_nc()
    in_maps = make_in_maps(inputs)
    res = run_bass_kernel_spmd(nc, in_maps, core_ids=list(range(NCORES)))
    out = np.concatenate([r['out'] for r in res.results], axis=0)
    return out.astype(np.float32)


if __name__ == "__main__":
    # smoke-build
    nc = build_bass()
    print("built ok:",
          sum(len(b.instructions) for f in nc.m.functions for b in f.blocks),
          "instructions")
